# revision 1
# baseline (speedup 1.0000x reference)
"""CRF (Viterbi decode + log-norm + sequence score) Bass kernel for TRN2.

Layout: batch rows on partitions (2 tiles of 128 rows per core, B=2048 over
8 cores => 256 rows/core). Scores tensor per step: [128, (kc,kp)] with kc
outer (stride-0 broadcast of alpha over kc; trans pre-transposed kc-major).

Backpointers are stored in "red space": bpRv[t][kc] = 31 - argmax_kp, with
ties broken like jnp.argmax (first/lowest kp) via max over (31-kp)*eq.
Traceback runs a per-step select-sum chain in red space.
"""
import numpy as np
import concourse.bass as bass
import concourse.bacc as bacc
import concourse.mybir as mybir
import concourse.tile as tile

f32 = mybir.dt.float32
i32 = mybir.dt.int32
u16 = mybir.dt.uint16
u8 = mybir.dt.uint8
Alu = mybir.AluOpType
Act = mybir.ActivationFunctionType
AX = mybir.AxisListType

K = 32
KK = K * K  # 1024


def build_program(T, rows=256, Tc=64):
    """Build the SPMD program for one core handling `rows` batch rows, T steps."""
    ntiles = rows // 128
    nchunks = T // Tc
    nc = bacc.Bacc("TRN2", target_bir_lowering=False, num_devices=8)

    # ---- DRAM I/O ----
    pot_d = nc.dram_tensor("pot", [rows, T, K], f32, kind="ExternalInput")
    transT_d = nc.dram_tensor("transT", [128, KK], f32, kind="ExternalInput")
    iotaR_d = nc.dram_tensor("iotaR", [128, KK], f32, kind="ExternalInput")
    iotaK_d = nc.dram_tensor("iotaK", [128, K], f32, kind="ExternalInput")
    identR_d = nc.dram_tensor("identR", [128, K], u16, kind="ExternalInput")
    umask_d = nc.dram_tensor("umask", [rows, T], f32, kind="ExternalInput")
    tagk_d = nc.dram_tensor("tagk", [rows, T], f32, kind="ExternalInput")
    histT_d = nc.dram_tensor("histT", [8, 128, rows], f32, kind="ExternalInput")
    taucol_d = nc.dram_tensor("taucol", [8, 128, 1], f32, kind="ExternalInput")

    otags_d = nc.dram_tensor("otags", [rows, T], i32, kind="ExternalOutput")
    obest_d = nc.dram_tensor("obest", [rows, 1], f32, kind="ExternalOutput")
    oll_d = nc.dram_tensor("oll", [rows, 1], f32, kind="ExternalOutput")

    with tile.TileContext(nc) as tc:
        with (
            tc.tile_pool(name="const", bufs=1) as cpool,
            tc.tile_pool(name="state", bufs=1) as spool,
            tc.tile_pool(name="potc", bufs=3) as potpool,
            tc.tile_pool(name="scr", bufs=2) as scr,
            tc.tile_pool(name="small", bufs=2) as sm,
            tc.tile_pool(name="psum", bufs=2, space="PSUM") as pp,
        ):
            # ---- constants ----
            transT_t = cpool.tile([128, KK], f32, tag="transT")
            iotaR_t = cpool.tile([128, KK], f32, tag="iotaR")
            iotaK_t = cpool.tile([128, K], f32, tag="iotaK")
            identR_t = cpool.tile([128, K], u16, tag="identR")
            nc.sync.dma_start(transT_t[:], transT_d[:])
            nc.sync.dma_start(iotaR_t[:], iotaR_d[:])
            nc.sync.dma_start(iotaK_t[:], iotaK_d[:])
            nc.sync.dma_start(identR_t[:], identR_d[:])

            # ---- per-tile persistent state ----
            alpha, alphaf, bpRv, umask_t, tagk_t, upart, tagsredf = [], [], [], [], [], [], []
            for q in range(ntiles):
                alpha.append(spool.tile([128, K], f32, tag=f"alpha{q}"))
                alphaf.append(spool.tile([128, K], f32, tag=f"alphaf{q}"))
                bpRv.append(spool.tile([128, T * K], u16, tag=f"bpRv{q}"))
                umask_t.append(spool.tile([128, T], f32, tag=f"umask{q}"))
                tagk_t.append(spool.tile([128, T], f32, tag=f"tagk{q}"))
                upart.append(spool.tile([128, nchunks], f32, tag=f"upart{q}"))
                tagsredf.append(spool.tile([128, T], f32, tag=f"tagsredf{q}"))
                rs = slice(q * 128, q * 128 + 128)
                nc.sync.dma_start(umask_t[q][:], umask_d[rs, :])
                nc.sync.dma_start(tagk_t[q][:], tagk_d[rs, :])
                # bpRv init to identR replicated over T slots
                nc.vector.tensor_copy(
                    bpRv[q][:].rearrange("p (t k) -> p t k", k=K),
                    identR_t[:].unsqueeze(1).broadcast_to([128, T, K]),
                )

            # ---- chunk 0 load + alpha init ----
            chunks = [[None] * nchunks for _ in range(ntiles)]
            for q in range(ntiles):
                chunks[q][0] = potpool.tile([128, Tc * K], f32, tag=f"potc{q}")
                nc.sync.dma_start(
                    chunks[q][0][:].rearrange("p (t k) -> p t k", k=K),
                    pot_d[q * 128 : q * 128 + 128, 0:Tc, :],
                )
                nc.vector.tensor_copy(alpha[q][:], chunks[q][0][:, 0:K])
                nc.vector.tensor_copy(alphaf[q][:], chunks[q][0][:, 0:K])

            def unary_chunk(q, c):
                """select-sum of pot[b,t,tag] over chunk c (invalid tag => 99 => no hit)."""
                e_u = scr.tile([128, Tc * K], f32, tag=f"eu{q}")
                junk = scr.tile([128, Tc * K], f32, tag=f"junku{q}")
                iota_b = iotaK_t[:].unsqueeze(1).broadcast_to([128, Tc, K])
                tag_b = (
                    tagk_t[q][:, c * Tc : (c + 1) * Tc].unsqueeze(2).broadcast_to([128, Tc, K])
                )
                nc.vector.tensor_tensor(
                    e_u[:].rearrange("p (t k) -> p t k", k=K), iota_b, tag_b, op=Alu.is_equal
                )
                nc.vector.scalar_tensor_tensor(
                    junk[:], e_u[:], 1.0, chunks[q][c][:],
                    op0=Alu.mult, op1=Alu.mult,
                    accum_out=upart[q][:, c : c + 1],
                )

            # ---- forward over time ----
            for t in range(1, T):
                c = t // Tc
                for q in range(ntiles):
                    if t % Tc == 0:
                        # finish unary for previous chunk, load this one
                        unary_chunk(q, c - 1)
                        chunks[q][c] = potpool.tile([128, Tc * K], f32, tag=f"potc{q}")
                        nc.sync.dma_start(
                            chunks[q][c][:].rearrange("p (t k) -> p t k", k=K),
                            pot_d[q * 128 : q * 128 + 128, c * Tc : (c + 1) * Tc, :],
                        )
                    potcol = chunks[q][c][:, (t % Tc) * K : (t % Tc + 1) * K]
                    mcol = umask_t[q][:, t : t + 1].unsqueeze(2).broadcast_to([128, 1, K])

                    # --- viterbi step ---
                    scv = scr.tile([128, KK], f32, tag=f"scv{q}")
                    alpha_b = alpha[q][:].unsqueeze(1).broadcast_to([128, K, K])
                    nc.vector.tensor_tensor(
                        scv[:].rearrange("p (a b) -> p a b", b=K),
                        alpha_b,
                        transT_t[:].rearrange("p (a b) -> p a b", b=K),
                        op=Alu.add,
                    )
                    Mv = sm.tile([128, K], f32, tag=f"Mv{q}")
                    nc.vector.reduce_max(
                        Mv[:], scv[:].rearrange("p (a b) -> p a b", b=K), axis=AX.X
                    )
                    Ev = scr.tile([128, KK], f32, tag=f"Ev{q}")
                    Mv_b = Mv[:].unsqueeze(2).broadcast_to([128, K, K])
                    nc.vector.tensor_tensor(
                        Ev[:].rearrange("p (a b) -> p a b", b=K),
                        scv[:].rearrange("p (a b) -> p a b", b=K),
                        Mv_b,
                        op=Alu.is_ge,
                    )
                    Wv = scr.tile([128, KK], f32, tag=f"Wv{q}")
                    nc.vector.tensor_tensor(Wv[:], Ev[:], iotaR_t[:], op=Alu.mult)
                    redf = sm.tile([128, K], f32, tag=f"redf{q}")
                    nc.vector.reduce_max(
                        redf[:], Wv[:].rearrange("p (a b) -> p a b", b=K), axis=AX.X
                    )
                    redu = sm.tile([128, K], u16, tag=f"redu{q}")
                    nc.vector.tensor_copy(redu[:], redf[:])
                    nc.vector.copy_predicated(
                        bpRv[q][:, t * K : (t + 1) * K].unsqueeze(1),
                        mcol,
                        redu[:].unsqueeze(1),
                    )
                    tmpv = sm.tile([128, K], f32, tag=f"tmpv{q}")
                    nc.vector.tensor_tensor(tmpv[:], Mv[:], potcol, op=Alu.add)
                    nc.vector.copy_predicated(
                        alpha[q][:].unsqueeze(1), mcol, tmpv[:].unsqueeze(1)
                    )

                    # --- log-norm step ---
                    nmf = sm.tile([128, 1], f32, tag=f"nmf{q}")
                    nc.vector.reduce_max(nmf[:], alphaf[q][:], axis=AX.X, negate=True)
                    scf = scr.tile([128, KK], f32, tag=f"scf{q}")
                    alphaf_b = alphaf[q][:].unsqueeze(1).broadcast_to([128, K, K])
                    nc.vector.tensor_tensor(
                        scf[:].rearrange("p (a b) -> p a b", b=K),
                        alphaf_b,
                        transT_t[:].rearrange("p (a b) -> p a b", b=K),
                        op=Alu.add,
                    )
                    exf = scr.tile([128, KK], f32, tag=f"exf{q}")
                    nc.scalar.activation(exf[:], scf[:], Act.Exp, bias=nmf[:, 0:1])
                    Sf = sm.tile([128, K], f32, tag=f"Sf{q}")
                    nc.vector.reduce_sum(
                        Sf[:], exf[:].rearrange("p (a b) -> p a b", b=K), axis=AX.X
                    )
                    Lf = sm.tile([128, K], f32, tag=f"Lf{q}")
                    nc.scalar.activation(Lf[:], Sf[:], Act.Log)
                    tmpf = sm.tile([128, K], f32, tag=f"tmpf{q}")
                    nc.vector.scalar_tensor_tensor(
                        tmpf[:], Lf[:], nmf[:, 0:1], potcol, op0=Alu.subtract, op1=Alu.add
                    )
                    nc.vector.copy_predicated(
                        alphaf[q][:].unsqueeze(1), mcol, tmpf[:].unsqueeze(1)
                    )

            # ---- epilogue per tile ----
            for q in range(ntiles):
                unary_chunk(q, nchunks - 1)
                rs = slice(q * 128, q * 128 + 128)

                best = sm.tile([128, 1], f32, tag=f"best{q}")
                nc.vector.reduce_max(best[:], alpha[q][:], axis=AX.X)
                nc.sync.dma_start(obest_d[rs, :], best[:])

                e32 = sm.tile([128, K], f32, tag=f"e32{q}")
                nc.vector.tensor_scalar(
                    e32[:], alpha[q][:], best[:, 0:1], None, op0=Alu.is_ge
                )
                w32 = sm.tile([128, K], f32, tag=f"w32{q}")
                nc.vector.tensor_tensor(w32[:], e32[:], iotaR_t[:, 0:K], op=Alu.mult)
                nc.vector.reduce_max(tagsredf[q][:, T - 1 : T], w32[:], axis=AX.X)

                # log_norm of final alphaf
                nm2 = sm.tile([128, 1], f32, tag=f"nm2{q}")
                nc.vector.reduce_max(nm2[:], alphaf[q][:], axis=AX.X, negate=True)
                ex2 = sm.tile([128, K], f32, tag=f"ex2{q}")
                nc.scalar.activation(ex2[:], alphaf[q][:], Act.Exp, bias=nm2[:, 0:1])
                s2 = sm.tile([128, 1], f32, tag=f"s2{q}")
                nc.vector.reduce_sum(s2[:], ex2[:], axis=AX.X)
                l2 = sm.tile([128, 1], f32, tag=f"l2{q}")
                nc.scalar.activation(l2[:], s2[:], Act.Log)
                lognorm = sm.tile([128, 1], f32, tag=f"lognorm{q}")
                nc.vector.tensor_scalar(
                    lognorm[:], l2[:], nm2[:, 0:1], None, op0=Alu.subtract
                )

                # unary total
                utot = sm.tile([128, 1], f32, tag=f"utot{q}")
                nc.vector.reduce_sum(utot[:], upart[q][:], axis=AX.X)

                # binary via histogram matvec on PE
                binp = pp.tile([128, 1], f32, tag=f"binp{q}")
                hts = []
                for cchunk in range(8):
                    ht = sm.tile([128, 128], f32, tag=f"ht{q}")
                    nc.sync.dma_start(ht[:], histT_d[cchunk, :, rs])
                    hts.append(ht)
                tcs = []
                for cchunk in range(8):
                    tcol = sm.tile([128, 1], f32, tag=f"tc{q}")
                    nc.sync.dma_start(tcol[:], taucol_d[cchunk, :, :])
                    tcs.append(tcol)
                for cchunk in range(8):
                    nc.tensor.matmul(
                        binp[:], hts[cchunk][:], tcs[cchunk][:],
                        start=(cchunk == 0), stop=(cchunk == 7),
                    )
                binv = sm.tile([128, 1], f32, tag=f"binv{q}")
                nc.vector.tensor_copy(binv[:], binp[:])

                ll = sm.tile([128, 1], f32, tag=f"ll{q}")
                nc.vector.scalar_tensor_tensor(
                    ll[:], utot[:], lognorm[:, 0:1], binv[:], op0=Alu.subtract, op1=Alu.add
                )
                nc.sync.dma_start(oll_d[rs, :], ll[:])

                # ---- traceback chain (red space) ----
                junk32 = sm.tile([128, K], f32, tag=f"junk32{q}")
                for t in range(T - 1, 0, -1):
                    echain = sm.tile([128, K], f32, tag=f"echain{q}")
                    nc.vector.tensor_scalar(
                        echain[:], iotaR_t[:, 0:K], tagsredf[q][:, t : t + 1], None,
                        op0=Alu.is_equal,
                    )
                    nc.vector.scalar_tensor_tensor(
                        junk32[:], echain[:], 1.0,
                        bpRv[q][:, t * K : (t + 1) * K],
                        op0=Alu.mult, op1=Alu.mult,
                        accum_out=tagsredf[q][:, t - 1 : t],
                    )
                otags_t = sm.tile([128, T], i32, tag=f"otags{q}")
                nc.vector.tensor_scalar(
                    otags_t[:], tagsredf[q][:], -1.0, float(K - 1),
                    op0=Alu.mult, op1=Alu.add,
                )
                nc.sync.dma_start(otags_d[rs, :], otags_t[:])

    nc.compile()
    return nc


def host_prepare(potentials, transitions, sequence_lengths, tag_indices, n_cores=8):
    """Shard + build aux tensors. Returns (in_maps, perm) where perm[core*rows+i] = original row."""
    B, T, Kk = potentials.shape
    rows = B // n_cores
    perm = np.arange(B)  # v1: plain contiguous sharding

    transT = np.ascontiguousarray(transitions.T).reshape(1, KK)  # (kc,kp) kc-major
    transT_rep = np.broadcast_to(transT, (128, KK)).copy().astype(np.float32)
    kp = np.arange(K)
    iotaR = np.broadcast_to(
        np.tile((K - 1 - kp).astype(np.float32), K)[None, :], (128, KK)
    ).copy()
    iotaK = np.broadcast_to(kp.astype(np.float32)[None, :], (128, K)).copy()
    identR = np.broadcast_to((K - 1 - kp).astype(np.uint16)[None, :], (128, K)).copy()
    taucol = transitions.reshape(KK).astype(np.float32).reshape(8, 128, 1)

    t_range = np.arange(T)
    umask_all = (t_range[None, :] < sequence_lengths[:, None]).astype(np.float32)
    tagk_all = np.where(umask_all > 0, tag_indices.astype(np.float32), 99.0)

    # binary histogram: hist[b, j] = count of (tag_t*K + tag_{t+1}) over t < len-1
    bmask = (t_range[None, : T - 1] < (sequence_lengths - 1)[:, None])
    pair = tag_indices[:, :-1].astype(np.int64) * K + tag_indices[:, 1:].astype(np.int64)
    hist = np.zeros((B, KK), dtype=np.float32)
    rows_idx = np.repeat(np.arange(B), T - 1)
    np.add.at(hist, (rows_idx, pair.reshape(-1)), bmask.reshape(-1).astype(np.float32))

    in_maps = []
    for core in range(n_cores):
        sl = perm[core * rows : (core + 1) * rows]
        histT = np.ascontiguousarray(hist[sl].T).reshape(8, 128, rows).astype(np.float32)
        in_maps.append(
            {
                "pot": np.ascontiguousarray(potentials[sl]).astype(np.float32),
                "transT": transT_rep,
                "iotaR": iotaR,
                "iotaK": iotaK,
                "identR": identR,
                "umask": np.ascontiguousarray(umask_all[sl]),
                "tagk": np.ascontiguousarray(tagk_all[sl]),
                "histT": histT,
                "taucol": taucol,
            }
        )
    return in_maps, perm


_NC_CACHE = {}


def _get_program(T):
    if T not in _NC_CACHE:
        _NC_CACHE[T] = build_program(T)
    return _NC_CACHE[T]


def kernel(potentials, transitions, sequence_lengths, tag_indices):
    """CRF forward/viterbi/score on 8 NeuronCores. Returns (decode_tags, best_score, log_likelihood)."""
    from concourse.bass_utils import run_bass_kernel_spmd

    potentials = np.ascontiguousarray(potentials, dtype=np.float32)
    transitions = np.ascontiguousarray(transitions, dtype=np.float32)
    sequence_lengths = np.ascontiguousarray(sequence_lengths, dtype=np.int32)
    tag_indices = np.ascontiguousarray(tag_indices, dtype=np.int32)
    B, T, Kk = potentials.shape

    nc = _get_program(T)
    in_maps, perm = host_prepare(potentials, transitions, sequence_lengths, tag_indices)
    res = run_bass_kernel_spmd(nc, in_maps, core_ids=list(range(8)))

    tags = np.concatenate([r["otags"] for r in res.results], axis=0)
    best = np.concatenate([r["obest"][:, 0] for r in res.results], axis=0)
    ll = np.concatenate([r["oll"][:, 0] for r in res.results], axis=0)
    inv = np.empty_like(perm)
    inv[perm] = np.arange(B)
    return tags[inv].astype(np.int32), best[inv].astype(np.float32), ll[inv].astype(np.float32)
